# revision 3
# baseline (speedup 1.0000x reference)
"""AttentionFusion kernel for 8 Trainium2 NeuronCores (v3).

Reference computation (B=2, C=256, H=W=64, N=8192 tokens = 2 modalities x 4096):
    x    = concat(flat(feat0), flat(feat1))        # [B, N, C]
    Q,K,V = x @ W{q,k,v}.T + b{q,k,v}
    attn = softmax(Q @ K.T / 16)
    out  = (attn @ V) @ Wo.T + bo                  # [B, N, C]
    out  = mean over modalities -> [B, HW, C] -> [B, C, H, W]

Sharding: 8 cores = (2 batches) x (4 query groups). Core (b, g) computes
queries {g*1024..(g+1)*1024} of each modality (2048 rows) for batch b, with
full K/V (8192 tokens) computed locally. The modality mean pairs rows within
a core, so there is no cross-core communication at all.

Everything is computed in "transposed" (feature-on-partition) layout; no
transposes needed anywhere.

Host-side folds (all exact linear algebra, done in fp32):
  - X, weights pre-cast to fp16 on the host (no on-device casts).
  - W~v = Wo @ Wv: the output projection is folded into V, so
    out = softmax(S) @ V~ + bo_eff with V~ = X @ W~v.T and
    bo_eff = bo + Wo @ bv (the V bias rides through softmax rows
    summing to 1). The on-device output projection disappears.
  - bq_eff = bq / 16 (score scale folded), bk dropped (softmax-invariant).
  - Softmax denominators: bc = 1/(twos.T @ sa) = 0.5/sums via a single
    all-2.0 stationary matmul (broadcasts column sums to 128 partitions
    in one shot), reciprocal_approx_fast, so the modality-mean 0.5 is
    free and exact.

Device schedule per core (PE-bound ~250us stream):
  - warmup matmuls keep the PE HAM busy during the initial DMA wait
  - phase Q / phase KV: projections, fp16, K evac on ACT, V evac on DVE
  - attention: per 1024-wide q-chunk, 64 k-tiles, software-pipelined
    2 deep (PE order S(kt+1) before O(kt)) so exp latency hides;
    softmax-sum accumulated in fp16 on DVE (2x mode).
"""

import numpy as np

B, C, H, W = 2, 256, 64, 64
HW = H * W            # 4096
NTOK = 2 * HW         # 8192 tokens per batch (2 modalities)
NQ = 2048             # q columns per core
P = 128
KT = NTOK // P        # 64 k-tiles
JT = NTOK // 1024     # 8 x-tiles of 1024 tokens
QCH = 1024            # q-chunk width (2 PSUM banks)
NCH = NQ // QCH       # 2 q-chunks per core (= modalities)
NCORES = 8

_compiled = {}


def _build():
    import concourse.bass as bass  # noqa: F401
    import concourse.mybir as mybir
    from concourse import bacc
    from concourse.tile import TileContext

    f32 = mybir.dt.float32
    f16 = mybir.dt.float16
    COPY = mybir.ActivationFunctionType.Copy
    EXP = mybir.ActivationFunctionType.Exp
    MULT = mybir.AluOpType.mult
    ADD = mybir.AluOpType.add

    nc = bacc.Bacc("TRN2", target_bir_lowering=False, debug=False,
                   num_devices=NCORES)

    xT = nc.dram_tensor("xT", [C, NTOK], f16, kind="ExternalInput")
    xTq = nc.dram_tensor("xTq", [C, NQ], f16, kind="ExternalInput")
    # wcat = [wqT | wkT | wvT] along the output axis
    wcat_d = nc.dram_tensor("wcat", [C, 3 * C], f16, kind="ExternalInput")
    # bcat = [bq_eff | bo_eff] columns
    bcat_d = nc.dram_tensor("bcat", [C, 2], f32, kind="ExternalInput")
    out_d = nc.dram_tensor("out", [C, QCH], f32, kind="ExternalOutput")

    with TileContext(nc) as tc:
        with tc.tile_pool(name="const", bufs=1) as cpool, \
             tc.tile_pool(name="kTp", bufs=1) as kTp, \
             tc.tile_pool(name="qTp", bufs=1) as qTp, \
             tc.tile_pool(name="Vp", bufs=1) as Vp:

            # scratch for PE warmup during the initial DMA wait
            wsc = cpool.tile([P, P], f16, tag="wsc")
            nc.vector.memset(wsc[:], 0.0)
            # all-2.0 stationary: one matmul broadcasts 2*colsum(sa) to
            # all 128 partitions -> reciprocal gives 0.5/sums directly
            twos = cpool.tile([P, P], f16, tag="twos")
            nc.vector.memset(twos[:], 2.0)

            wb_sb = []   # [wq0, wq1, wk0, wk1, wv0, wv1] each [P, C] f16
            for h in range(2):
                t = cpool.tile([P, 3 * C], f16, tag=f"wb{h}")
                nc.sync.dma_start(t[:], wcat_d.ap()[h * P:(h + 1) * P, :])
                wb_sb.append(t)
            wq_sb = [wb_sb[h][:, 0:C] for h in range(2)]
            wk_sb = [wb_sb[h][:, C:2 * C] for h in range(2)]
            wv_sb = [wb_sb[h][:, 2 * C:3 * C] for h in range(2)]

            bb_sb = []
            for h in range(2):
                t = cpool.tile([P, 2], f32, tag=f"bb{h}")
                nc.sync.dma_start(t[:], bcat_d.ap()[h * P:(h + 1) * P, :])
                bb_sb.append(t)
            bq_sb = [bb_sb[h][:, 0:1] for h in range(2)]
            bo_sb = [bb_sb[h][:, 1:2] for h in range(2)]

            # persistent activations
            kT = [kTp.tile([P, NTOK], f16, tag=f"kT{h}", name=f"kT{h}")
                  for h in range(2)]
            qT = [qTp.tile([P, NQ], f16, tag=f"qT{h}", name=f"qT{h}")
                  for h in range(2)]
            Vb = Vp.tile([P, KT * C], f16, tag="Vb")  # [k-part, kt*256 + c]

            with tc.tile_pool(name="xcp", bufs=4) as xcp, \
                 tc.tile_pool(name="sps", bufs=2, space="PSUM") as sps, \
                 tc.tile_pool(name="ops", bufs=2, space="PSUM") as ops, \
                 tc.tile_pool(name="pp", bufs=4) as pp, \
                 tc.tile_pool(name="sap", bufs=2) as sap, \
                 tc.tile_pool(name="bcp", bufs=2) as bcp, \
                 tc.tile_pool(name="nnp", bufs=4) as nnp, \
                 tc.tile_pool(name="osb", bufs=2) as osb:

                # ---- PE warmup: ~40 N=128 matmuls (~4.3us cold) flip the
                # HAM to 8/8 while the first DMAs are still in flight
                wps = sps.tile([P, P], f32, tag="sp")
                for _ in range(40):
                    nc.tensor.matmul(wps[:], wsc[:], wsc[:],
                                     start=True, stop=True)

                # ---- phase Q: Q^T = (Wq^T.T @ Xq^T)/16 + bq/16 ----
                for mod in range(2):
                    xq = []
                    for h in range(2):
                        t = xcp.tile([P, 1024], f16, tag=f"xc{h}")
                        nc.sync.dma_start(
                            t[:], xTq.ap()[h * P:(h + 1) * P,
                                           mod * 1024:(mod + 1) * 1024])
                        xq.append(t)
                    for ch in range(2):
                        qp = sps.tile([P, 1024], f32, tag="sp")
                        for hc in range(2):
                            o = qp[:, hc * 512:(hc + 1) * 512]
                            nc.tensor.matmul(
                                o, wq_sb[0][:, ch * P:(ch + 1) * P],
                                xq[0][:, hc * 512:(hc + 1) * 512],
                                start=True, stop=False)
                            nc.tensor.matmul(
                                o, wq_sb[1][:, ch * P:(ch + 1) * P],
                                xq[1][:, hc * 512:(hc + 1) * 512],
                                start=False, stop=True)
                        nc.vector.tensor_scalar(
                            qT[ch][:, mod * 1024:(mod + 1) * 1024], qp[:],
                            1.0 / 16.0, bq_sb[ch], MULT, ADD)

                # ---- phase KV: stream X^T, compute K^T and V~ ----
                for j in range(JT):
                    xc = []
                    for h in range(2):
                        t = xcp.tile([P, 1024], f16, tag=f"xc{h}")
                        nc.sync.dma_start(
                            t[:], xT.ap()[h * P:(h + 1) * P,
                                          j * 1024:(j + 1) * 1024])
                        xc.append(t)
                    for ch in range(2):
                        kp = sps.tile([P, 1024], f32, tag="sp")
                        for hc in range(2):
                            o = kp[:, hc * 512:(hc + 1) * 512]
                            nc.tensor.matmul(
                                o, wk_sb[0][:, ch * P:(ch + 1) * P],
                                xc[0][:, hc * 512:(hc + 1) * 512],
                                start=True, stop=False)
                            nc.tensor.matmul(
                                o, wk_sb[1][:, ch * P:(ch + 1) * P],
                                xc[1][:, hc * 512:(hc + 1) * 512],
                                start=False, stop=True)
                        nc.scalar.activation(
                            kT[ch][:, j * 1024:(j + 1) * 1024], kp[:], COPY)
                    for g in range(2):
                        vp = ops.tile([P, 1024], f32, tag="op")
                        for t in range(4):
                            tok = g * 512 + t * P
                            o = vp[:, t * 256:(t + 1) * 256]
                            nc.tensor.matmul(
                                o, xc[0][:, tok:tok + P], wv_sb[0],
                                start=True, stop=False)
                            nc.tensor.matmul(
                                o, xc[1][:, tok:tok + P], wv_sb[1],
                                start=False, stop=True)
                        kt0 = j * 8 + g * 4
                        nc.vector.tensor_copy(
                            Vb[:, kt0 * C:(kt0 + 4) * C], vp[:])

                # ---- phase 2: attention per q-chunk (= modality) ----
                def s_exp(chunk, kt):
                    """S^T tile matmuls + exp; returns the p tile."""
                    qb = chunk * QCH
                    sp = sps.tile([P, QCH], f32, tag="sp", name=f"sp{kt}")
                    for hc in range(2):
                        o = sp[:, hc * 512:(hc + 1) * 512]
                        nc.tensor.matmul(
                            o, kT[0][:, kt * P:(kt + 1) * P],
                            qT[0][:, qb + hc * 512:qb + (hc + 1) * 512],
                            start=True, stop=False)
                        nc.tensor.matmul(
                            o, kT[1][:, kt * P:(kt + 1) * P],
                            qT[1][:, qb + hc * 512:qb + (hc + 1) * 512],
                            start=False, stop=True)
                    p = pp.tile([P, QCH], f16, tag="p", name=f"p{kt}")
                    nc.scalar.activation(p[:], sp[:], EXP)
                    return p

                def o_acc(o_ps, sa, p, kt):
                    """accumulate O^T += V-tile.T @ P^T, sa += p."""
                    first, last = kt == 0, kt == KT - 1
                    for ch in range(2):
                        for hc in range(2):
                            nc.tensor.matmul(
                                o_ps[ch][:, hc * 512:(hc + 1) * 512],
                                Vb[:, kt * C + ch * P:kt * C + (ch + 1) * P],
                                p[:, hc * 512:(hc + 1) * 512],
                                start=first, stop=last)
                    nc.vector.tensor_add(sa[:], sa[:], p[:])

                stash = None
                for chunk in range(NCH):
                    o_ps = [ops.tile([P, QCH], f32, tag="op",
                                     name=f"o{chunk}_{ch}")
                            for ch in range(2)]
                    sa = sap.tile([P, QCH], f16, tag="sa")
                    nc.vector.memset(sa[:], 0.0)

                    # software pipeline, 2 deep: PE order is
                    # S(0), S(1), O(0), S(2), O(1), ..., S(63), O(62), O(63)
                    p_prev = s_exp(chunk, 0)
                    for kt in range(1, KT):
                        p_cur = s_exp(chunk, kt)
                        o_acc(o_ps, sa, p_prev, kt - 1)
                        p_prev = p_cur
                    o_acc(o_ps, sa, p_prev, KT - 1)

                    # bc = 0.5/sums on all 128 partitions:
                    # twos.T @ sa = 2*sums broadcast, then 1/x
                    bc_ps = sps.tile([P, QCH], f32, tag="sp")
                    for hc in range(2):
                        nc.tensor.matmul(
                            bc_ps[:, hc * 512:(hc + 1) * 512], twos[:],
                            sa[:, hc * 512:(hc + 1) * 512],
                            start=True, stop=True)
                    bc = bcp.tile([P, QCH], f32, tag="bc")
                    nc.vector.reciprocal_approx_fast(bc[:], bc_ps[:])

                    if chunk == 0:
                        stash = []
                        for ch in range(2):
                            t = nnp.tile([P, QCH], f32, tag="nn")
                            nc.vector.tensor_mul(t[:], o_ps[ch][:], bc[:])
                            stash.append(t)
                    else:
                        for ch in range(2):
                            t = nnp.tile([P, QCH], f32, tag="nn")
                            nc.vector.tensor_mul(t[:], o_ps[ch][:], bc[:])
                            ot = osb.tile([P, QCH], f32, tag="os")
                            # out = (n1 + bo_eff) + n0 in one fused op
                            nc.vector.scalar_tensor_tensor(
                                ot[:], t[:], bo_sb[ch], stash[ch][:],
                                ADD, ADD)
                            nc.sync.dma_start(
                                out_d.ap()[ch * P:(ch + 1) * P, :], ot[:])

    nc.compile()
    return nc


def _get_compiled():
    if "nc" not in _compiled:
        _compiled["nc"] = _build()
    return _compiled["nc"]


def kernel(feat0, feat1, Wq, bq, Wk, bk, Wv, bv, Wo, bo):
    from concourse.bass_utils import run_bass_kernel_spmd

    feat0 = np.asarray(feat0, dtype=np.float32)
    feat1 = np.asarray(feat1, dtype=np.float32)
    Wq = np.asarray(Wq, dtype=np.float32)
    Wk = np.asarray(Wk, dtype=np.float32)
    Wv = np.asarray(Wv, dtype=np.float32)
    Wo = np.asarray(Wo, dtype=np.float32)
    bq = np.asarray(bq, dtype=np.float32)
    bv = np.asarray(bv, dtype=np.float32)
    bo = np.asarray(bo, dtype=np.float32)

    wqT = Wq.T
    wkT = Wk.T
    # fold output projection into V: V~ = X @ (Wo @ Wv).T
    wvT = (Wo @ Wv).T
    wcat = np.ascontiguousarray(
        np.concatenate([wqT, wkT, wvT], axis=1)).astype(np.float16)
    bq_eff = bq / 16.0
    bo_eff = bo + Wo @ bv
    bcat = np.ascontiguousarray(np.stack([bq_eff, bo_eff], axis=1))

    xT_all = [
        np.ascontiguousarray(
            np.concatenate([feat0[b].reshape(C, HW), feat1[b].reshape(C, HW)],
                           axis=1)).astype(np.float16)
        for b in range(B)
    ]

    in_maps = []
    for core in range(NCORES):
        b, g = core // 4, core % 4
        cols0 = slice(g * 1024, (g + 1) * 1024)
        cols1 = slice(HW + g * 1024, HW + (g + 1) * 1024)
        xTq = np.ascontiguousarray(
            np.concatenate([xT_all[b][:, cols0], xT_all[b][:, cols1]],
                           axis=1))
        in_maps.append({
            "xT": xT_all[b], "xTq": xTq,
            "wcat": wcat, "bcat": bcat,
        })

    global _last_in_maps
    _last_in_maps = in_maps

    nc = _get_compiled()
    res = run_bass_kernel_spmd(nc, in_maps, core_ids=list(range(NCORES)))

    full = np.empty((B, C, HW), dtype=np.float32)
    for core in range(NCORES):
        b, g = core // 4, core % 4
        full[b][:, g * 1024:(g + 1) * 1024] = res.results[core]["out"]
    return full.reshape(B, C, H, W)


# revision 10
# speedup vs baseline: 1.4370x; 1.4370x over previous
"""AttentionFusion kernel for 8 Trainium2 NeuronCores (v3).

Reference computation (B=2, C=256, H=W=64, N=8192 tokens = 2 modalities x 4096):
    x    = concat(flat(feat0), flat(feat1))        # [B, N, C]
    Q,K,V = x @ W{q,k,v}.T + b{q,k,v}
    attn = softmax(Q @ K.T / 16)
    out  = (attn @ V) @ Wo.T + bo                  # [B, N, C]
    out  = mean over modalities -> [B, HW, C] -> [B, C, H, W]

Sharding: 8 cores = (2 batches) x (4 query groups). Core (b, g) computes
queries {g*1024..(g+1)*1024} of each modality (2048 rows) for batch b, with
full K/V (8192 tokens) computed locally. The modality mean pairs rows within
a core, so there is no cross-core communication at all.

Everything is computed in "transposed" (feature-on-partition) layout; no
transposes needed anywhere.

Host-side folds (all exact linear algebra, done in fp32):
  - X, weights pre-cast to fp16 on the host (no on-device casts).
  - W~v = Wo @ Wv: the output projection is folded into V, so
    out = softmax(S) @ V~ + bo_eff with V~ = X @ W~v.T and
    bo_eff = bo + Wo @ bv (the V bias rides through softmax rows
    summing to 1). The on-device output projection disappears.
  - bq_eff = bq / 16 (score scale folded), bk dropped (softmax-invariant).
  - Softmax denominators: bc = 1/(twos.T @ sa) = 0.5/sums via a single
    all-2.0 stationary matmul (broadcasts column sums to 128 partitions
    in one shot), reciprocal_approx_fast, so the modality-mean 0.5 is
    free and exact.

Device schedule per core (PE-bound ~250us stream):
  - warmup matmuls keep the PE HAM busy during the initial DMA wait
  - phase Q / phase KV: projections, fp16, K evac on ACT, V evac on DVE
  - attention: per 1024-wide q-chunk, 64 k-tiles, software-pipelined
    2 deep (PE order S(kt+1) before O(kt)) so exp latency hides;
    softmax-sum accumulated in fp16 on DVE (2x mode).
"""

import numpy as np

B, C, H, W = 2, 256, 64, 64
HW = H * W            # 4096
NTOK = 2 * HW         # 8192 tokens per batch (2 modalities)
NQ = 2048             # q columns per core
P = 128
KT = NTOK // P        # 64 k-tiles
JT = NTOK // 1024     # 8 x-tiles of 1024 tokens
QCH = 1024            # q-chunk width (2 PSUM banks)
NCH = NQ // QCH       # 2 q-chunks per core (= modalities)
NCORES = 8

_compiled = {}


def _build():
    import concourse.bass as bass  # noqa: F401
    import concourse.mybir as mybir
    from concourse import bacc
    from concourse.tile import TileContext

    f32 = mybir.dt.float32
    f16 = mybir.dt.float16
    f8 = mybir.dt.float8e4
    DR = mybir.MatmulPerfMode.DoubleRow
    COPY = mybir.ActivationFunctionType.Copy
    EXP = mybir.ActivationFunctionType.Exp
    MULT = mybir.AluOpType.mult
    ADD = mybir.AluOpType.add

    nc = bacc.Bacc("TRN2", target_bir_lowering=False, debug=False,
                   num_devices=NCORES)

    xT = nc.dram_tensor("xT", [C, NTOK], f16, kind="ExternalInput")
    xTq = nc.dram_tensor("xTq", [C, NQ], f16, kind="ExternalInput")
    # wcat = [wqT | wkT | wvT] along the output axis
    wcat_d = nc.dram_tensor("wcat", [C, 3 * C], f16, kind="ExternalInput")
    # bcat = [bq_eff | bo_eff] columns
    bcat_d = nc.dram_tensor("bcat", [C, 2], f32, kind="ExternalInput")
    out_d = nc.dram_tensor("out", [C, QCH], f32, kind="ExternalOutput")

    with TileContext(nc) as tc:
        with tc.tile_pool(name="const", bufs=1) as cpool, \
             tc.tile_pool(name="kTp", bufs=1) as kTp, \
             tc.tile_pool(name="qTp", bufs=1) as qTp, \
             tc.tile_pool(name="Vp", bufs=1) as Vp:

            # scratch for PE warmup during the initial DMA wait
            wsc = cpool.tile([P, P], f16, tag="wsc")
            nc.vector.memset(wsc[:], 0.0)
            # all-2.0 stationary: one matmul broadcasts 2*colsum(sa) to
            # all 128 partitions -> reciprocal gives 0.5/sums directly
            twos = cpool.tile([P, P], f16, tag="twos")
            nc.vector.memset(twos[:], 2.0)

            wb_sb = []   # [wq0, wq1, wk0, wk1, wv0, wv1] each [P, C] f16
            for h in range(2):
                t = cpool.tile([P, 3 * C], f16, tag=f"wb{h}")
                nc.sync.dma_start(t[:], wcat_d.ap()[h * P:(h + 1) * P, :])
                wb_sb.append(t)
            wq_sb = [wb_sb[h][:, 0:C] for h in range(2)]
            wk_sb = [wb_sb[h][:, C:2 * C] for h in range(2)]
            wv_sb = [wb_sb[h][:, 2 * C:3 * C] for h in range(2)]

            bb_sb = []
            for h in range(2):
                t = cpool.tile([P, 2], f32, tag=f"bb{h}")
                nc.sync.dma_start(t[:], bcat_d.ap()[h * P:(h + 1) * P, :])
                bb_sb.append(t)
            bq_sb = [bb_sb[h][:, 0:1] for h in range(2)]
            bo_sb = [bb_sb[h][:, 1:2] for h in range(2)]

            # persistent activations. K^T/Q^T in fp8e4 (O(1)-scale values;
            # the /16 score scale is applied via exp's free affine), laid
            # out [part, c-half, col] so a DoubleRow matmul contracts all
            # 256 channels in one instruction.
            kT8 = kTp.tile([P, 2, NTOK], f8, tag="kT8", name="kT8")
            qT8 = qTp.tile([P, 2, NQ], f8, tag="qT8", name="qT8")
            Vb = Vp.tile([P, KT * C], f16, tag="Vb")  # [k-part, kt*256 + c]

            with tc.tile_pool(name="xcp", bufs=4) as xcp, \
                 tc.tile_pool(name="sps", bufs=2, space="PSUM") as sps, \
                 tc.tile_pool(name="ops", bufs=2, space="PSUM") as ops, \
                 tc.tile_pool(name="pp", bufs=4) as pp, \
                 tc.tile_pool(name="sap", bufs=2) as sap, \
                 tc.tile_pool(name="bcp", bufs=2) as bcp, \
                 tc.tile_pool(name="nnp", bufs=4) as nnp, \
                 tc.tile_pool(name="osb", bufs=2) as osb:

                # ---- PE warmup: N=128 matmuls (~5us cold) flip the
                # HAM to 8/8 while the first DMAs are still in flight
                wps = sps.tile([P, P], f32, tag="sp")
                for _ in range(16):
                    nc.tensor.matmul(wps[:], wsc[:], wsc[:],
                                     start=True, stop=True)

                # ---- phase Q: Q^T = (Wq^T.T @ Xq^T)/16 + bq/16 ----
                for mod in range(2):
                    xq = []
                    for h in range(2):
                        t = xcp.tile([P, 1024], f16, tag=f"xc{h}")
                        nc.sync.dma_start(
                            t[:], xTq.ap()[h * P:(h + 1) * P,
                                           mod * 1024:(mod + 1) * 1024])
                        xq.append(t)
                    for ch in range(2):
                        qp = sps.tile([P, 1024], f32, tag="sp")
                        for hc in range(2):
                            o = qp[:, hc * 512:(hc + 1) * 512]
                            nc.tensor.matmul(
                                o, wq_sb[0][:, ch * P:(ch + 1) * P],
                                xq[0][:, hc * 512:(hc + 1) * 512],
                                start=True, stop=False)
                            nc.tensor.matmul(
                                o, wq_sb[1][:, ch * P:(ch + 1) * P],
                                xq[1][:, hc * 512:(hc + 1) * 512],
                                start=False, stop=True)
                        nc.vector.tensor_scalar(
                            qT8[:, ch, mod * 1024:(mod + 1) * 1024], qp[:],
                            bq_sb[ch], None, ADD)

                # ---- phase KV: stream X^T, compute K^T and V~ ----
                for j in range(JT):
                    xc = []
                    for h in range(2):
                        t = xcp.tile([P, 1024], f16, tag=f"xc{h}")
                        nc.sync.dma_start(
                            t[:], xT.ap()[h * P:(h + 1) * P,
                                          j * 1024:(j + 1) * 1024])
                        xc.append(t)
                    for ch in range(2):
                        kp = sps.tile([P, 1024], f32, tag="sp")
                        for hc in range(2):
                            o = kp[:, hc * 512:(hc + 1) * 512]
                            nc.tensor.matmul(
                                o, wk_sb[0][:, ch * P:(ch + 1) * P],
                                xc[0][:, hc * 512:(hc + 1) * 512],
                                start=True, stop=False)
                            nc.tensor.matmul(
                                o, wk_sb[1][:, ch * P:(ch + 1) * P],
                                xc[1][:, hc * 512:(hc + 1) * 512],
                                start=False, stop=True)
                        nc.scalar.activation(
                            kT8[:, ch, j * 1024:(j + 1) * 1024], kp[:], COPY)
                    for g in range(2):
                        vp = ops.tile([P, 1024], f32, tag="op")
                        for t in range(4):
                            tok = g * 512 + t * P
                            o = vp[:, t * 256:(t + 1) * 256]
                            nc.tensor.matmul(
                                o, xc[0][:, tok:tok + P], wv_sb[0],
                                start=True, stop=False)
                            nc.tensor.matmul(
                                o, xc[1][:, tok:tok + P], wv_sb[1],
                                start=False, stop=True)
                        kt0 = j * 8 + g * 4
                        nc.vector.tensor_copy(
                            Vb[:, kt0 * C:(kt0 + 4) * C], vp[:])

                # ---- phase 2: attention per q-chunk (= modality) ----
                def s_exp(chunk, kt):
                    """S^T tile DoubleRow matmuls + exp; returns the p tile.

                    One fp8 DoubleRow matmul contracts all 256 channels:
                    out[m,n] = sum_{p,i} kT8[p,i,m] * qT8[p,i,n]."""
                    qb = chunk * QCH
                    sp = sps.tile([P, QCH], f32, tag="sp", name=f"sp{kt}")
                    for hc in range(2):
                        nc.tensor.matmul(
                            sp[:, hc * 512:(hc + 1) * 512],
                            kT8[:, :, kt * P:(kt + 1) * P],
                            qT8[:, :, qb + hc * 512:qb + (hc + 1) * 512],
                            start=True, stop=True, perf_mode=DR)
                    p = pp.tile([P, QCH], f16, tag="p", name=f"p{kt}")
                    # scores are Q.K (O(256) scale); /16 via the free affine
                    nc.scalar.activation(p[:], sp[:], EXP, scale=1.0 / 16.0)
                    return p

                def o_acc(o_ps, sa, p, kt):
                    """accumulate O^T += V-tile.T @ P^T, sa += p."""
                    first, last = kt == 0, kt == KT - 1
                    for ch in range(2):
                        for hc in range(2):
                            nc.tensor.matmul(
                                o_ps[ch][:, hc * 512:(hc + 1) * 512],
                                Vb[:, kt * C + ch * P:kt * C + (ch + 1) * P],
                                p[:, hc * 512:(hc + 1) * 512],
                                start=first, stop=last)
                    nc.vector.tensor_add(sa[:], sa[:], p[:])

                stash = None
                for chunk in range(NCH):
                    o_ps = [ops.tile([P, QCH], f32, tag="op",
                                     name=f"o{chunk}_{ch}")
                            for ch in range(2)]
                    sa = sap.tile([P, QCH], f16, tag="sa")
                    nc.vector.memset(sa[:], 0.0)

                    # software pipeline, 2 deep: PE order is
                    # S(0), S(1), O(0), S(2), O(1), ..., S(63), O(62), O(63)
                    p_prev = s_exp(chunk, 0)
                    for kt in range(1, KT):
                        p_cur = s_exp(chunk, kt)
                        o_acc(o_ps, sa, p_prev, kt - 1)
                        p_prev = p_cur
                    o_acc(o_ps, sa, p_prev, KT - 1)

                    # bc = 0.5/sums on all 128 partitions:
                    # twos.T @ sa = 2*sums broadcast, then 1/x
                    bc_ps = sps.tile([P, QCH], f32, tag="sp")
                    for hc in range(2):
                        nc.tensor.matmul(
                            bc_ps[:, hc * 512:(hc + 1) * 512], twos[:],
                            sa[:, hc * 512:(hc + 1) * 512],
                            start=True, stop=True)
                    bc = bcp.tile([P, QCH], f32, tag="bc")
                    nc.vector.reciprocal_approx_fast(bc[:], bc_ps[:])

                    if chunk == 0:
                        stash = []
                        for ch in range(2):
                            t = nnp.tile([P, QCH], f32, tag="nn")
                            nc.vector.tensor_mul(t[:], o_ps[ch][:], bc[:])
                            stash.append(t)
                    else:
                        for ch in range(2):
                            t = nnp.tile([P, QCH], f32, tag="nn")
                            nc.vector.tensor_mul(t[:], o_ps[ch][:], bc[:])
                            ot = osb.tile([P, QCH], f32, tag="os")
                            # out = (n1 + bo_eff) + n0 in one fused op
                            nc.vector.scalar_tensor_tensor(
                                ot[:], t[:], bo_sb[ch], stash[ch][:],
                                ADD, ADD)
                            nc.sync.dma_start(
                                out_d.ap()[ch * P:(ch + 1) * P, :], ot[:])

    nc.compile()
    return nc


def _get_compiled():
    if "nc" not in _compiled:
        _compiled["nc"] = _build()
    return _compiled["nc"]


def kernel(feat0, feat1, Wq, bq, Wk, bk, Wv, bv, Wo, bo):
    from concourse.bass_utils import run_bass_kernel_spmd

    feat0 = np.asarray(feat0, dtype=np.float32)
    feat1 = np.asarray(feat1, dtype=np.float32)
    Wq = np.asarray(Wq, dtype=np.float32)
    Wk = np.asarray(Wk, dtype=np.float32)
    Wv = np.asarray(Wv, dtype=np.float32)
    Wo = np.asarray(Wo, dtype=np.float32)
    bq = np.asarray(bq, dtype=np.float32)
    bv = np.asarray(bv, dtype=np.float32)
    bo = np.asarray(bo, dtype=np.float32)

    wqT = Wq.T
    wkT = Wk.T
    # fold output projection into V: V~ = X @ (Wo @ Wv).T
    wvT = (Wo @ Wv).T
    wcat = np.ascontiguousarray(
        np.concatenate([wqT, wkT, wvT], axis=1)).astype(np.float16)
    bq_eff = bq  # Q kept at O(1) scale; /16 applied inside exp
    bo_eff = bo + Wo @ bv
    bcat = np.ascontiguousarray(np.stack([bq_eff, bo_eff], axis=1))

    xT_all = [
        np.ascontiguousarray(
            np.concatenate([feat0[b].reshape(C, HW), feat1[b].reshape(C, HW)],
                           axis=1)).astype(np.float16)
        for b in range(B)
    ]

    in_maps = []
    for core in range(NCORES):
        b, g = core // 4, core % 4
        cols0 = slice(g * 1024, (g + 1) * 1024)
        cols1 = slice(HW + g * 1024, HW + (g + 1) * 1024)
        xTq = np.ascontiguousarray(
            np.concatenate([xT_all[b][:, cols0], xT_all[b][:, cols1]],
                           axis=1))
        in_maps.append({
            "xT": xT_all[b], "xTq": xTq,
            "wcat": wcat, "bcat": bcat,
        })

    global _last_in_maps
    _last_in_maps = in_maps

    nc = _get_compiled()
    res = run_bass_kernel_spmd(nc, in_maps, core_ids=list(range(NCORES)))

    full = np.empty((B, C, HW), dtype=np.float32)
    for core in range(NCORES):
        b, g = core // 4, core % 4
        full[b][:, g * 1024:(g + 1) * 1024] = res.results[core]["out"]
    return full.reshape(B, C, H, W)
